# revision 8
# baseline (speedup 1.0000x reference)
"""Trainium2 Bass kernel for CarlosSelfAttention (B=2, T=2048, C=1024, H=16).

Sharding: tensor-parallel over heads. 8 cores x 2 heads each.
Each core computes q/k/v projections for its 2 heads, RoPE, causal
attention, and a partial out-projection against its 128 columns of Wo.
The host sums the 8 partial outputs (the TP all-reduce) and adds the
output bias plus the (v-bias @ Wo.T) correction term.

All on-chip layouts are "transposed" ([dim, token]) so every matmul
contraction lands on the partition axis:
  xT   [1024, 4096]   (input, replicated)
  qT/kT[128, 4096]    rows = [h0-even dims, h0-odd, h1-even, h1-odd]
  vT   2 x [64, 4096] rows = plain head dims
  S^T  [k-tile 128, q-chunk 512] via PE, exp'd on ScalarE from PSUM
  P@V  col-packed (h0 -> psum rows 0:63, h1 -> 64:127), sums via
       ones-matmul, normalization by reciprocal broadcast.
  out  y_part [4096, 1024] = OT.T @ WoT via PE, DMA'd from PSUM.
"""

import os
import numpy as np

import concourse.bass as bass
import concourse.tile as tile
from concourse import bacc, mybir
from concourse.bass_utils import run_bass_kernel_spmd

F32 = mybir.dt.float32
F32R = mybir.dt.float32r
AF = mybir.ActivationFunctionType

B, T, C, H, HD = 2, 2048, 1024, 16, 64
NCORES = 8
TB = B * T          # 4096
QCH = 512           # q-chunk (moving dim)
NQC = T // QCH      # 4 q-chunks per batch
NKT = T // 128      # 16 k-tiles per batch
NTC = TB // QCH     # 8 t-chunks for the projections
NCT = C // 128      # 8 contraction tiles

_PROG_CACHE: dict = {}


def _emit(tc, mode, dram):
    nc = tc.nc
    from contextlib import ExitStack

    xT, wT, bqk, cosT, sinS, woT, y = (
        dram["xT"], dram["wT"], dram["bqk"], dram["cosT"], dram["sinS"],
        dram["woT"], dram["y"])
    maskT = dram.get("maskT")

    with ExitStack() as ctx:
        constp = ctx.enter_context(tc.tile_pool(name="const", bufs=1))
        pers = ctx.enter_context(tc.tile_pool(name="pers", bufs=1))

        # ---- constants ----
        wsb = constp.tile([128, NCT, 384], F32)
        nc.sync.dma_start(wsb[:].bitcast(F32R),
                          wT[:].rearrange("(a p) m -> p a m", p=128).bitcast(F32R))
        cos_sb = constp.tile([128, T], F32)
        nc.sync.dma_start(cos_sb[:], cosT[:])
        sin_sb = constp.tile([128, T], F32)
        nc.sync.dma_start(sin_sb[:], sinS[:])
        bqk_sb = constp.tile([128, 2], F32)
        nc.sync.dma_start(bqk_sb[:], bqk[:])
        wo_sb = constp.tile([128, C], F32)
        nc.sync.dma_start(wo_sb[:].bitcast(F32R), woT[:].bitcast(F32R))
        ones_stg = constp.tile([128, 1], F32)
        nc.vector.memset(ones_stg[:], 1.0)
        ones_sb = constp.tile([128, 1], F32)
        nc.vector.tensor_copy(ones_sb[:].bitcast(F32R), ones_stg[:])
        id64 = constp.tile([64, 64], F32)
        nc.vector.memset(id64[:], 1.0)
        nc.gpsimd.affine_select(
            out=id64[:], in_=id64[:], compare_op=mybir.AluOpType.is_equal,
            fill=0.0, base=0, channel_multiplier=1, pattern=[[-1, 64]])

        # ---- persistent activations ----
        qT = pers.tile([128, TB], F32)
        kT = pers.tile([128, TB], F32)
        vTf = pers.tile([128, TB], F32)
        vT1 = pers.tile([64, TB], F32)
        # V in [t, hd] layout per (b, h): [128, 16 tiles x 64]
        Vsb = [[pers.tile([128, NKT * HD], F32, name=f"Vsb{b}{h}")
                for h in range(2)] for b in range(B)]
        OT = [pers.tile([128, T], F32, name=f"OTb{b}") for b in range(B)]

        # ---- phase 1: qkv projection ----
        with tc.tile_pool(name="xp", bufs=6) as xp, \
             tc.tile_pool(name="psqkv", bufs=3, space="PSUM") as psqkv:
            for tci in range(NTC):
                ts = slice(tci * QCH, (tci + 1) * QCH)
                xtiles = []
                for ct in range(NCT):
                    xt = xp.tile([128, QCH], F32, tag="x")
                    nc.sync.dma_start(
                        xt[:].bitcast(F32R),
                        xT[ct * 128:(ct + 1) * 128, ts].bitcast(F32R))
                    xtiles.append(xt)
                for g in range(3):
                    ps = psqkv.tile([128, QCH], F32, tag="ps")
                    for ct in range(NCT):
                        nc.tensor.matmul(
                            ps[:],
                            wsb[:, ct, g * 128:(g + 1) * 128].bitcast(F32R),
                            xtiles[ct][:].bitcast(F32R),
                            start=(ct == 0), stop=(ct == NCT - 1))
                    if g == 0:
                        nc.scalar.activation(qT[:, ts].bitcast(F32R), ps[:],
                                             AF.Identity, bias=bqk_sb[:, 0:1])
                    elif g == 1:
                        nc.scalar.activation(kT[:, ts].bitcast(F32R), ps[:],
                                             AF.Identity, bias=bqk_sb[:, 1:2])
                    else:
                        nc.scalar.activation(vTf[:, ts], ps[:], AF.Copy)
                        nc.sync.dma_start(vT1[:, ts], vTf[64:128, ts])

        # ---- phase 2: RoPE on qT, kT (in place) ----
        with tc.tile_pool(name="swp", bufs=2) as swpp, \
             tc.tile_pool(name="rtmp", bufs=2) as rtp:
            for zt in (qT, kT):
                swp = swpp.tile([128, TB], F32, tag="swp")
                for h in range(2):
                    o = h * 64
                    nc.sync.dma_start(swp[o:o + 32, :], zt[o + 32:o + 64, :])
                    nc.sync.dma_start(swp[o + 32:o + 64, :], zt[o:o + 32, :])
                for b in range(B):
                    bs = slice(b * T, (b + 1) * T)
                    tmp = rtp.tile([128, T], F32, tag="rt")
                    nc.vector.tensor_mul(tmp[:], swp[:, bs], sin_sb[:])
                    nc.vector.tensor_mul(zt[:, bs].bitcast(F32R), zt[:, bs],
                                         cos_sb[:])
                    nc.vector.tensor_add(zt[:, bs].bitcast(F32R), zt[:, bs],
                                         tmp[:])

        # ---- phase 3: V transposes into [t, hd] tiles ----
        with tc.tile_pool(name="pstr", bufs=2, space="PSUM") as pstr:
            for b in range(B):
                for h, vt in ((0, vTf), (1, vT1)):
                    for tt in range(NKT):
                        pst = pstr.tile([128, 64], F32, tag="tr")
                        nc.tensor.transpose(
                            pst[:], vt[0:64, b * T + tt * 128: b * T + (tt + 1) * 128],
                            id64[:])
                        nc.vector.tensor_copy(
                            Vsb[b][h][:, tt * HD:(tt + 1) * HD].bitcast(F32R),
                            pst[:])

        # ---- phase 4: attention ----
        with tc.tile_pool(name="pss", bufs=3, space="PSUM") as pss, \
             tc.tile_pool(name="pso", bufs=1, space="PSUM") as pso, \
             tc.tile_pool(name="pssum", bufs=1, space="PSUM") as pssum, \
             tc.tile_pool(name="ptp", bufs=6) as ptp, \
             tc.tile_pool(name="mbp", bufs=4) as mbp, \
             tc.tile_pool(name="smol", bufs=4) as smol, \
             tc.tile_pool(name="bcp", bufs=4) as bcp:
            for b in range(B):
                for qc in range(NQC):
                    nk = 4 * (qc + 1) if mode == "causal" else NKT
                    qs = slice(b * T + qc * QCH, b * T + (qc + 1) * QCH)
                    psO0 = pso.tile([128, QCH], F32, tag="o0")
                    psO1 = pso.tile([128, QCH], F32, tag="o1")
                    psSum = pssum.tile([1, 2 * QCH], F32, tag="sm")
                    for kt in range(nk):
                        ks = slice(b * T + kt * 128, b * T + (kt + 1) * 128)
                        st, sp = (kt == 0), (kt == nk - 1)
                        psS0 = pss.tile([128, QCH], F32, tag="s")
                        psS1 = pss.tile([128, QCH], F32, tag="s")
                        nc.tensor.matmul(psS0[:], kT[0:64, ks].bitcast(F32R),
                                         qT[0:64, qs].bitcast(F32R),
                                         start=True, stop=True)
                        nc.tensor.matmul(psS1[:], kT[64:128, ks].bitcast(F32R),
                                         qT[64:128, qs].bitcast(F32R),
                                         start=True, stop=True)
                        pt0 = ptp.tile([128, QCH], F32, tag="pt")
                        pt1 = ptp.tile([128, QCH], F32, tag="pt")
                        nc.scalar.activation(pt0[:].bitcast(F32R), psS0[:],
                                             AF.Exp)
                        nc.scalar.activation(pt1[:].bitcast(F32R), psS1[:],
                                             AF.Exp)
                        if mode == "causal" and kt >= 4 * qc:
                            base = qc * QCH - kt * 128
                            for pt in (pt0, pt1):
                                nc.gpsimd.affine_select(
                                    out=pt[:].bitcast(F32R),
                                    in_=pt[:].bitcast(F32R),
                                    compare_op=mybir.AluOpType.is_ge,
                                    fill=0.0, base=base,
                                    channel_multiplier=-1,
                                    pattern=[[1, QCH]])
                        elif mode == "bias":
                            mt = mbp.tile([128, QCH], F32, tag="mb")
                            nc.sync.dma_start(
                                mt[:], maskT[kt * 128:(kt + 1) * 128,
                                             qc * QCH:(qc + 1) * QCH])
                            nc.vector.tensor_mul(pt0[:].bitcast(F32R), pt0[:],
                                                 mt[:])
                            nc.vector.tensor_mul(pt1[:].bitcast(F32R), pt1[:],
                                                 mt[:])
                        nc.tensor.matmul(
                            psO0[0:64, :],
                            Vsb[b][0][:, kt * HD:(kt + 1) * HD].bitcast(F32R),
                            pt0[:].bitcast(F32R), start=st, stop=sp,
                            tile_position=(0, 0))
                        nc.tensor.matmul(
                            psO1[0:64, :],
                            Vsb[b][1][:, kt * HD:(kt + 1) * HD].bitcast(F32R),
                            pt1[:].bitcast(F32R), start=st, stop=sp)
                        nc.tensor.matmul(psSum[:, 0:QCH],
                                         ones_sb[:].bitcast(F32R),
                                         pt0[:].bitcast(F32R),
                                         start=st, stop=sp)
                        nc.tensor.matmul(psSum[:, QCH:2 * QCH],
                                         ones_sb[:].bitcast(F32R),
                                         pt1[:].bitcast(F32R),
                                         start=st, stop=sp)
                    oqs = slice(qc * QCH, (qc + 1) * QCH)
                    rc0 = smol.tile([1, QCH], F32, tag="rc")
                    rc1 = smol.tile([1, QCH], F32, tag="rc")
                    nc.vector.reciprocal(rc0[:], psSum[:, 0:QCH])
                    nc.vector.reciprocal(rc1[:], psSum[:, QCH:2 * QCH])
                    bc0 = bcp.tile([128, QCH], F32, tag="bc")
                    bc1 = bcp.tile([128, QCH], F32, tag="bc")
                    nc.gpsimd.partition_broadcast(bc0[:], rc0[:])
                    nc.gpsimd.partition_broadcast(bc1[:], rc1[:])
                    nc.vector.tensor_mul(OT[b][0:64, oqs].bitcast(F32R),
                                         psO0[0:64, :], bc0[0:64, :])
                    otmp = bcp.tile([64, QCH], F32, tag="otmp")
                    nc.vector.tensor_mul(otmp[:], psO1[0:64, :], bc1[0:64, :])
                    nc.sync.dma_start(OT[b][64:128, oqs].bitcast(F32R),
                                      otmp[:].bitcast(F32R))

        # ---- phase 5: out projection ----
        with tc.tile_pool(name="psy", bufs=4, space="PSUM") as psy, \
             tc.tile_pool(name="ybp", bufs=4) as ybp:
            for b in range(B):
                for tt in range(NKT):
                    for ncol in range(2):
                        ps = psy.tile([128, QCH], F32, tag="y")
                        nc.tensor.matmul(
                            ps[:],
                            OT[b][:, tt * 128:(tt + 1) * 128].bitcast(F32R),
                            wo_sb[:, ncol * QCH:(ncol + 1) * QCH].bitcast(F32R),
                            start=True, stop=True)
                        yb = ybp.tile([128, QCH], F32, tag="yb")
                        nc.any.tensor_copy(yb[:], ps[:])
                        nc.sync.dma_start(
                            y[b * T + tt * 128: b * T + (tt + 1) * 128,
                              ncol * QCH:(ncol + 1) * QCH], yb[:])


def _build_program(mode):
    if mode in _PROG_CACHE:
        return _PROG_CACHE[mode]
    nc = bacc.Bacc("TRN2", target_bir_lowering=False, debug=False,
                   num_devices=NCORES)
    dram = {
        "xT": nc.dram_tensor("xT", [C, TB], F32, kind="ExternalInput").ap(),
        "wT": nc.dram_tensor("wT", [C, 384], F32, kind="ExternalInput").ap(),
        "bqk": nc.dram_tensor("bqk", [128, 2], F32, kind="ExternalInput").ap(),
        "cosT": nc.dram_tensor("cosT", [128, T], F32, kind="ExternalInput").ap(),
        "sinS": nc.dram_tensor("sinS", [128, T], F32, kind="ExternalInput").ap(),
        "woT": nc.dram_tensor("woT", [128, C], F32, kind="ExternalInput").ap(),
        "y": nc.dram_tensor("y", [TB, C], F32, kind="ExternalOutput").ap(),
    }
    if mode == "bias":
        dram["maskT"] = nc.dram_tensor("maskT", [T, T], F32,
                                       kind="ExternalInput").ap()
    with tile.TileContext(nc) as tc:
        _emit(tc, mode, dram)
    nc.compile()
    _PROG_CACHE[mode] = (nc, dram)
    return nc, dram


def _rope_tables():
    inv_freq = 1.0 / (10000.0 ** (np.arange(0, HD, 2, dtype=np.float64) / HD))
    freqs = np.arange(T, dtype=np.float64)[:, None] * inv_freq[None, :]
    cos = np.concatenate([np.cos(freqs), np.cos(freqs)], axis=-1)  # [T, 64]
    sin = np.concatenate([np.sin(freqs), np.sin(freqs)], axis=-1)
    cE = cos[:, 0::2].T  # [32, T] rows i -> dim 2i
    cO = cos[:, 1::2].T
    sE = sin[:, 0::2].T
    sO = sin[:, 1::2].T
    cosT = np.concatenate([cE, cO, cE, cO], axis=0).astype(np.float32)
    sinS = np.concatenate([-sE, sO, -sE, sO], axis=0).astype(np.float32)
    return np.ascontiguousarray(cosT), np.ascontiguousarray(sinS)


def kernel(x, mask, Wqkv, bqkv, Wo, bo):
    x = np.asarray(x, dtype=np.float32)
    mask = np.asarray(mask)
    Wqkv = np.asarray(Wqkv, dtype=np.float32)
    bqkv = np.asarray(bqkv, dtype=np.float32)
    Wo = np.asarray(Wo, dtype=np.float32)
    bo = np.asarray(bo, dtype=np.float32)

    mb = mask.reshape(T, T)
    if np.array_equal(mb != 0, np.tril(np.ones((T, T), dtype=bool))):
        mode = "causal"
    elif np.all(mb != 0):
        mode = "dense"
    else:
        mode = "bias"

    nc, dram = _build_program(mode)

    xTn = np.ascontiguousarray(x.reshape(TB, C).T)
    cosT, sinS = _rope_tables()
    scale = 1.0 / np.sqrt(np.float32(HD))

    evens = np.arange(0, HD, 2)
    odds = evens + 1

    in_maps = []
    for c in range(NCORES):
        h0, h1 = 2 * c, 2 * c + 1
        qrows = np.concatenate([h0 * HD + evens, h0 * HD + odds,
                                h1 * HD + evens, h1 * HD + odds])
        krows = C + qrows
        vrows = np.concatenate([2 * C + h0 * HD + np.arange(HD),
                                2 * C + h1 * HD + np.arange(HD)])
        wq = Wqkv[qrows, :] * scale
        wk = Wqkv[krows, :]
        wv = Wqkv[vrows, :]
        wT = np.ascontiguousarray(np.concatenate([wq, wk, wv], axis=0).T)
        bqk = np.stack([bqkv[qrows] * scale, bqkv[krows]], axis=1)
        woT = np.ascontiguousarray(Wo[:, 128 * c:128 * (c + 1)].T)
        im = {
            "xT": xTn, "wT": wT,
            "bqk": np.ascontiguousarray(bqk, dtype=np.float32),
            "cosT": cosT, "sinS": sinS, "woT": woT,
        }
        if mode == "bias":
            im["maskT"] = np.ascontiguousarray(
                (mb != 0).astype(np.float32).T)
        in_maps.append(im)

    res = run_bass_kernel_spmd(nc, in_maps, core_ids=list(range(NCORES)))
    y = np.zeros((TB, C), dtype=np.float32)
    for c in range(NCORES):
        y += res.results[c]["y"]
    bv = bqkv[2 * C:3 * C]
    y += (bo + bv @ Wo.T)[None, :]
    return y.reshape(B, T, C)


# revision 15
# speedup vs baseline: 1.1668x; 1.1668x over previous
"""Trainium2 Bass kernel for CarlosSelfAttention (B=2, T=2048, C=1024, H=16).

Sharding: tensor-parallel over heads. 8 cores x 2 heads each.
Each core computes q/k/v projections for its 2 heads, RoPE, causal
attention, and a partial out-projection against its 128 columns of Wo.
The host sums the 8 partial outputs (the TP all-reduce) and adds the
output bias plus the (v-bias @ Wo.T) correction term.

All on-chip layouts are "transposed" ([dim, token]) so every matmul
contraction lands on the partition axis:
  xT   [1024, 4096]   (input, replicated)
  qT/kT[128, 4096]    rows = [h0-even dims, h0-odd, h1-even, h1-odd]
  vT   2 x [64, 4096] rows = plain head dims
  S^T  [k-tile 128, q-chunk 512] via PE, exp'd on ScalarE from PSUM
  P@V  col-packed (h0 -> psum rows 0:63, h1 -> 64:127), sums via
       ones-matmul, normalization by reciprocal broadcast.
  out  y_part [4096, 1024] = OT.T @ WoT via PE, DMA'd from PSUM.
"""

import os
import numpy as np

import concourse.bass as bass
import concourse.tile as tile
from concourse import bacc, mybir
from concourse.bass_utils import run_bass_kernel_spmd

F32 = mybir.dt.float32
F32R = mybir.dt.float32r
AF = mybir.ActivationFunctionType

B, T, C, H, HD = 2, 2048, 1024, 16, 64
NCORES = 8
TB = B * T          # 4096
QCH = 512           # q-chunk (moving dim)
NQC = T // QCH      # 4 q-chunks per batch
NKT = T // 128      # 16 k-tiles per batch
NTC = TB // QCH     # 8 t-chunks for the projections
NCT = C // 128      # 8 contraction tiles

_PROG_CACHE: dict = {}


def _emit(tc, mode, dram):
    nc = tc.nc
    from contextlib import ExitStack

    xT, wT, bqk, cosT, sinS, woT, y = (
        dram["xT"], dram["wT"], dram["bqk"], dram["cosT"], dram["sinS"],
        dram["woT"], dram["y"])
    maskT = dram.get("maskT")

    with ExitStack() as ctx:
        constp = ctx.enter_context(tc.tile_pool(name="const", bufs=1))
        pers = ctx.enter_context(tc.tile_pool(name="pers", bufs=1))

        # ---- constants ----
        wsb = constp.tile([128, NCT, 384], F32)
        nc.sync.dma_start(wsb[:].bitcast(F32R),
                          wT[:].rearrange("(a p) m -> p a m", p=128).bitcast(F32R))
        cos_sb = constp.tile([128, T], F32)
        nc.sync.dma_start(cos_sb[:], cosT[:])
        sin_sb = constp.tile([128, T], F32)
        nc.sync.dma_start(sin_sb[:], sinS[:])
        bqk_sb = constp.tile([128, 2], F32)
        nc.sync.dma_start(bqk_sb[:], bqk[:])
        wo_sb = constp.tile([128, C], F32)
        nc.sync.dma_start(wo_sb[:].bitcast(F32R), woT[:].bitcast(F32R))
        ones16 = constp.tile([128, NKT], F32)
        nc.vector.memset(ones16[:], 1.0)
        id64 = constp.tile([64, 64], F32)
        nc.vector.memset(id64[:], 1.0)
        nc.gpsimd.affine_select(
            out=id64[:], in_=id64[:], compare_op=mybir.AluOpType.is_equal,
            fill=0.0, base=0, channel_multiplier=1, pattern=[[-1, 64]])

        # ---- persistent activations ----
        qT = pers.tile([128, TB], F32)
        kT = pers.tile([128, TB], F32)
        vTf = pers.tile([128, TB], F32)
        vT1 = pers.tile([64, TB], F32)
        # V in [t, hd|ones] layout per (b, h): [128, 16 tiles x 65]
        Vsb = [[pers.tile([128, NKT * (HD + 1)], F32, name=f"Vsb{b}{h}")
                for h in range(2)] for b in range(B)]
        OT = [pers.tile([128, T], F32, name=f"OTb{b}") for b in range(B)]

        # ---- phase 1: qkv projection ----
        with tc.tile_pool(name="xp", bufs=6) as xp, \
             tc.tile_pool(name="psqkv", bufs=3, space="PSUM") as psqkv:
            for tci in range(NTC):
                ts = slice(tci * QCH, (tci + 1) * QCH)
                xtiles = []
                for ct in range(NCT):
                    xt = xp.tile([128, QCH], F32, tag="x")
                    nc.sync.dma_start(
                        xt[:].bitcast(F32R),
                        xT[ct * 128:(ct + 1) * 128, ts].bitcast(F32R))
                    xtiles.append(xt)
                for g in range(3):
                    ps = psqkv.tile([128, QCH], F32, tag="ps")
                    for ct in range(NCT):
                        nc.tensor.matmul(
                            ps[:],
                            wsb[:, ct, g * 128:(g + 1) * 128].bitcast(F32R),
                            xtiles[ct][:].bitcast(F32R),
                            start=(ct == 0), stop=(ct == NCT - 1))
                    if g == 0:
                        nc.scalar.activation(qT[:, ts].bitcast(F32R), ps[:],
                                             AF.Identity, bias=bqk_sb[:, 0:1])
                    elif g == 1:
                        nc.scalar.activation(kT[:, ts].bitcast(F32R), ps[:],
                                             AF.Identity, bias=bqk_sb[:, 1:2])
                    else:
                        nc.scalar.activation(vTf[:, ts], ps[:], AF.Copy)
                        nc.sync.dma_start(vT1[:, ts], vTf[64:128, ts])

        # ---- phase 2: RoPE on qT, kT (in place) ----
        with tc.tile_pool(name="swp", bufs=2) as swpp, \
             tc.tile_pool(name="rtmp", bufs=2) as rtp:
            for zt in (qT, kT):
                swp = swpp.tile([128, TB], F32, tag="swp")
                for h in range(2):
                    o = h * 64
                    nc.sync.dma_start(swp[o:o + 32, :], zt[o + 32:o + 64, :])
                    nc.sync.dma_start(swp[o + 32:o + 64, :], zt[o:o + 32, :])
                for b in range(B):
                    bs = slice(b * T, (b + 1) * T)
                    tmp = rtp.tile([128, T], F32, tag="rt")
                    nc.vector.tensor_mul(tmp[:], swp[:, bs], sin_sb[:])
                    nc.vector.tensor_mul(zt[:, bs].bitcast(F32R), zt[:, bs],
                                         cos_sb[:])
                    nc.vector.tensor_add(zt[:, bs].bitcast(F32R), zt[:, bs],
                                         tmp[:])

        # ---- phase 3: V transposes into [t, hd|1] tiles (ones col fused) ----
        with tc.tile_pool(name="pstr", bufs=2, space="PSUM") as pstr:
            for b in range(B):
                for h, vt in ((0, vTf), (1, vT1)):
                    # ones column at col 64 of each 65-wide tile
                    vov = Vsb[b][h][:].rearrange("p (t c) -> p t c", c=HD + 1)
                    nc.vector.tensor_copy(vov[:, :, HD:HD + 1].bitcast(F32R),
                                          ones16[:])
                    for tt in range(NKT):
                        pst = pstr.tile([128, 64], F32, tag="tr")
                        nc.tensor.transpose(
                            pst[:], vt[0:64, b * T + tt * 128: b * T + (tt + 1) * 128],
                            id64[:])
                        nc.vector.tensor_copy(
                            Vsb[b][h][:, tt * (HD + 1):tt * (HD + 1) + HD]
                            .bitcast(F32R),
                            pst[:])

        # ---- phase 4: attention (software-pipelined) ----
        PIPE = 3
        with tc.tile_pool(name="pss", bufs=2, space="PSUM") as pss, \
             tc.tile_pool(name="pso", bufs=2, space="PSUM") as pso, \
             tc.tile_pool(name="ptp", bufs=PIPE + 2) as ptp, \
             tc.tile_pool(name="mbp", bufs=4) as mbp, \
             tc.tile_pool(name="smol", bufs=4) as smol, \
             tc.tile_pool(name="bcp", bufs=4) as bcp:
            for b in range(B):
                for qc in range(NQC):
                    nk = 4 * (qc + 1) if mode == "causal" else NKT
                    qs = slice(b * T + qc * QCH, b * T + (qc + 1) * QCH)
                    psO0 = pso.tile([65, QCH], F32, tag="o0")
                    psO1 = pso.tile([65, QCH], F32, tag="o1")
                    pts = {}

                    def emit_pv(j, nk=nk, b=b, pts=pts, psO0=psO0, psO1=psO1):
                        st, sp = (j == 0), (j == nk - 1)
                        pt = pts.pop(j)
                        nc.tensor.matmul(
                            psO0[:],
                            Vsb[b][0][:, j * (HD + 1):(j + 1) * (HD + 1)]
                            .bitcast(F32R),
                            pt[:, 0:QCH].bitcast(F32R), start=st, stop=sp)
                        nc.tensor.matmul(
                            psO1[:],
                            Vsb[b][1][:, j * (HD + 1):(j + 1) * (HD + 1)]
                            .bitcast(F32R),
                            pt[:, QCH:2 * QCH].bitcast(F32R), start=st, stop=sp)

                    for kt in range(nk):
                        ks = slice(b * T + kt * 128, b * T + (kt + 1) * 128)
                        psS = pss.tile([128, 2 * QCH], F32, tag="s")
                        nc.tensor.matmul(psS[:, 0:QCH],
                                         kT[0:64, ks].bitcast(F32R),
                                         qT[0:64, qs].bitcast(F32R),
                                         start=True, stop=True)
                        nc.tensor.matmul(psS[:, QCH:2 * QCH],
                                         kT[64:128, ks].bitcast(F32R),
                                         qT[64:128, qs].bitcast(F32R),
                                         start=True, stop=True)
                        pt = ptp.tile([128, 2 * QCH], F32, tag="pt")
                        nc.scalar.activation(pt[:].bitcast(F32R), psS[:],
                                             AF.Exp)
                        if mode == "causal" and kt >= 4 * qc:
                            base = qc * QCH - kt * 128
                            ptv = pt[:].rearrange("p (h q) -> p h q", q=QCH)
                            nc.gpsimd.affine_select(
                                out=ptv.bitcast(F32R),
                                in_=ptv.bitcast(F32R),
                                compare_op=mybir.AluOpType.is_ge,
                                fill=0.0, base=base,
                                channel_multiplier=-1,
                                pattern=[[0, 2], [1, QCH]])
                        elif mode == "bias":
                            mt = mbp.tile([128, QCH], F32, tag="mb")
                            nc.sync.dma_start(
                                mt[:], maskT[kt * 128:(kt + 1) * 128,
                                             qc * QCH:(qc + 1) * QCH])
                            nc.vector.tensor_mul(pt[:, 0:QCH].bitcast(F32R),
                                                 pt[:, 0:QCH], mt[:])
                            nc.vector.tensor_mul(pt[:, QCH:2 * QCH]
                                                 .bitcast(F32R),
                                                 pt[:, QCH:2 * QCH], mt[:])
                        pts[kt] = pt
                        if kt >= PIPE:
                            emit_pv(kt - PIPE)
                    for j in range(max(0, nk - PIPE), nk):
                        emit_pv(j)

                    # normalize + evict; sum(exp) in row 64 of psO*
                    oqs = slice(qc * QCH, (qc + 1) * QCH)
                    for h, psO in ((0, psO0), (1, psO1)):
                        rw = smol.tile([65, QCH], F32, tag="rw")
                        nc.scalar.activation(rw[64:65, :], psO[64:65, :],
                                             AF.Copy)
                        rz = smol.tile([1, QCH], F32, tag="rz")
                        nc.sync.dma_start(rz[:], rw[64:65, :])
                        rr = smol.tile([1, QCH], F32, tag="rr")
                        nc.vector.reciprocal(rr[:], rz[:])
                        bc = bcp.tile([128, QCH], F32, tag="bc")
                        nc.gpsimd.partition_broadcast(bc[:], rr[:])
                        if h == 0:
                            nc.vector.tensor_mul(OT[b][0:64, oqs]
                                                 .bitcast(F32R),
                                                 psO[0:64, :], bc[0:64, :])
                        else:
                            otmp = bcp.tile([64, QCH], F32, tag="otmp")
                            nc.vector.tensor_mul(otmp[:], psO[0:64, :],
                                                 bc[0:64, :])
                            nc.sync.dma_start(OT[b][64:128, oqs].bitcast(F32R),
                                              otmp[:].bitcast(F32R))

        # ---- phase 5: out projection ----
        with tc.tile_pool(name="psy", bufs=4, space="PSUM") as psy, \
             tc.tile_pool(name="ybp", bufs=4) as ybp:
            for b in range(B):
                for tt in range(NKT):
                    for ncol in range(2):
                        ps = psy.tile([128, QCH], F32, tag="y")
                        nc.tensor.matmul(
                            ps[:],
                            OT[b][:, tt * 128:(tt + 1) * 128].bitcast(F32R),
                            wo_sb[:, ncol * QCH:(ncol + 1) * QCH].bitcast(F32R),
                            start=True, stop=True)
                        yb = ybp.tile([128, QCH], F32, tag="yb")
                        nc.vector.tensor_copy(yb[:], ps[:])
                        nc.sync.dma_start(
                            y[b * T + tt * 128: b * T + (tt + 1) * 128,
                              ncol * QCH:(ncol + 1) * QCH], yb[:])


def _build_program(mode):
    if mode in _PROG_CACHE:
        return _PROG_CACHE[mode]
    nc = bacc.Bacc("TRN2", target_bir_lowering=False, debug=False,
                   num_devices=NCORES)
    dram = {
        "xT": nc.dram_tensor("xT", [C, TB], F32, kind="ExternalInput").ap(),
        "wT": nc.dram_tensor("wT", [C, 384], F32, kind="ExternalInput").ap(),
        "bqk": nc.dram_tensor("bqk", [128, 2], F32, kind="ExternalInput").ap(),
        "cosT": nc.dram_tensor("cosT", [128, T], F32, kind="ExternalInput").ap(),
        "sinS": nc.dram_tensor("sinS", [128, T], F32, kind="ExternalInput").ap(),
        "woT": nc.dram_tensor("woT", [128, C], F32, kind="ExternalInput").ap(),
        "y": nc.dram_tensor("y", [TB, C], F32, kind="ExternalOutput").ap(),
    }
    if mode == "bias":
        dram["maskT"] = nc.dram_tensor("maskT", [T, T], F32,
                                       kind="ExternalInput").ap()
    with tile.TileContext(nc) as tc:
        _emit(tc, mode, dram)
    nc.compile()
    _PROG_CACHE[mode] = (nc, dram)
    return nc, dram


def _rope_tables():
    inv_freq = 1.0 / (10000.0 ** (np.arange(0, HD, 2, dtype=np.float64) / HD))
    freqs = np.arange(T, dtype=np.float64)[:, None] * inv_freq[None, :]
    cos = np.concatenate([np.cos(freqs), np.cos(freqs)], axis=-1)  # [T, 64]
    sin = np.concatenate([np.sin(freqs), np.sin(freqs)], axis=-1)
    cE = cos[:, 0::2].T  # [32, T] rows i -> dim 2i
    cO = cos[:, 1::2].T
    sE = sin[:, 0::2].T
    sO = sin[:, 1::2].T
    cosT = np.concatenate([cE, cO, cE, cO], axis=0).astype(np.float32)
    sinS = np.concatenate([-sE, sO, -sE, sO], axis=0).astype(np.float32)
    return np.ascontiguousarray(cosT), np.ascontiguousarray(sinS)


def kernel(x, mask, Wqkv, bqkv, Wo, bo):
    x = np.asarray(x, dtype=np.float32)
    mask = np.asarray(mask)
    Wqkv = np.asarray(Wqkv, dtype=np.float32)
    bqkv = np.asarray(bqkv, dtype=np.float32)
    Wo = np.asarray(Wo, dtype=np.float32)
    bo = np.asarray(bo, dtype=np.float32)

    mb = mask.reshape(T, T)
    if np.array_equal(mb != 0, np.tril(np.ones((T, T), dtype=bool))):
        mode = "causal"
    elif np.all(mb != 0):
        mode = "dense"
    else:
        mode = "bias"

    nc, dram = _build_program(mode)

    xTn = np.ascontiguousarray(x.reshape(TB, C).T)
    cosT, sinS = _rope_tables()
    scale = 1.0 / np.sqrt(np.float32(HD))

    evens = np.arange(0, HD, 2)
    odds = evens + 1

    in_maps = []
    for c in range(NCORES):
        h0, h1 = 2 * c, 2 * c + 1
        qrows = np.concatenate([h0 * HD + evens, h0 * HD + odds,
                                h1 * HD + evens, h1 * HD + odds])
        krows = C + qrows
        vrows = np.concatenate([2 * C + h0 * HD + np.arange(HD),
                                2 * C + h1 * HD + np.arange(HD)])
        wq = Wqkv[qrows, :] * scale
        wk = Wqkv[krows, :]
        wv = Wqkv[vrows, :]
        wT = np.ascontiguousarray(np.concatenate([wq, wk, wv], axis=0).T)
        bqk = np.stack([bqkv[qrows] * scale, bqkv[krows]], axis=1)
        woT = np.ascontiguousarray(Wo[:, 128 * c:128 * (c + 1)].T)
        im = {
            "xT": xTn, "wT": wT,
            "bqk": np.ascontiguousarray(bqk, dtype=np.float32),
            "cosT": cosT, "sinS": sinS, "woT": woT,
        }
        if mode == "bias":
            im["maskT"] = np.ascontiguousarray(
                (mb != 0).astype(np.float32).T)
        in_maps.append(im)

    res = run_bass_kernel_spmd(nc, in_maps, core_ids=list(range(NCORES)))
    y = np.zeros((TB, C), dtype=np.float32)
    for c in range(NCORES):
        y += res.results[c]["y"]
    bv = bqkv[2 * C:3 * C]
    y += (bo + bv @ Wo.T)[None, :]
    return y.reshape(B, T, C)


# revision 16
# speedup vs baseline: 1.4313x; 1.2267x over previous
"""Trainium2 Bass kernel for CarlosSelfAttention (B=2, T=2048, C=1024, H=16).

Sharding: tensor-parallel over heads. 8 cores x 2 heads each.
Each core computes q/k/v projections for its 2 heads, RoPE, causal
attention, and a partial out-projection against its 128 columns of Wo.
The host sums the 8 partial outputs (the TP all-reduce) and adds the
output bias plus the (v-bias @ Wo.T) correction term.

All on-chip layouts are "transposed" ([dim, token]) so every matmul
contraction lands on the partition axis:
  xT   [1024, 4096]   (input, replicated)
  qT/kT[128, 4096]    rows = [h0-even dims, h0-odd, h1-even, h1-odd]
  vT   2 x [64, 4096] rows = plain head dims
  S^T  [k-tile 128, q-chunk 512] via PE, exp'd on ScalarE from PSUM
  P@V  col-packed (h0 -> psum rows 0:63, h1 -> 64:127), sums via
       ones-matmul, normalization by reciprocal broadcast.
  out  y_part [4096, 1024] = OT.T @ WoT via PE, DMA'd from PSUM.
"""

import os
import numpy as np

import concourse.bass as bass
import concourse.tile as tile
from concourse import bacc, mybir
from concourse.bass_utils import run_bass_kernel_spmd

F32 = mybir.dt.float32
F32R = mybir.dt.float32r
AF = mybir.ActivationFunctionType

B, T, C, H, HD = 2, 2048, 1024, 16, 64
NCORES = 8
TB = B * T          # 4096
QCH = 512           # q-chunk (moving dim)
NQC = T // QCH      # 4 q-chunks per batch
NKT = T // 128      # 16 k-tiles per batch
NTC = TB // QCH     # 8 t-chunks for the projections
NCT = C // 128      # 8 contraction tiles

_PROG_CACHE: dict = {}


def _emit(tc, mode, dram):
    nc = tc.nc
    from contextlib import ExitStack

    xT, wT, bqk, cosT, sinS, woT, y = (
        dram["xT"], dram["wT"], dram["bqk"], dram["cosT"], dram["sinS"],
        dram["woT"], dram["y"])
    maskT = dram.get("maskT")

    with ExitStack() as ctx:
        constp = ctx.enter_context(tc.tile_pool(name="const", bufs=1))
        pers = ctx.enter_context(tc.tile_pool(name="pers", bufs=1))

        # ---- constants ----
        wsb = constp.tile([128, NCT, 384], F32)
        nc.sync.dma_start(wsb[:].bitcast(F32R),
                          wT[:].rearrange("(a p) m -> p a m", p=128).bitcast(F32R))
        cos_sb = constp.tile([128, T], F32)
        nc.sync.dma_start(cos_sb[:], cosT[:])
        sin_sb = constp.tile([128, T], F32)
        nc.sync.dma_start(sin_sb[:], sinS[:])
        bqk_sb = constp.tile([128, 2], F32)
        nc.sync.dma_start(bqk_sb[:], bqk[:])
        wo_sb = constp.tile([128, C], F32)
        nc.sync.dma_start(wo_sb[:].bitcast(F32R), woT[:].bitcast(F32R))
        ones16 = constp.tile([128, NKT], F32)
        nc.vector.memset(ones16[:], 1.0)
        id64 = constp.tile([64, 64], F32)
        nc.vector.memset(id64[:], 1.0)
        nc.gpsimd.affine_select(
            out=id64[:], in_=id64[:], compare_op=mybir.AluOpType.is_equal,
            fill=0.0, base=0, channel_multiplier=1, pattern=[[-1, 64]])

        # ---- persistent activations ----
        qT = pers.tile([128, TB], F32)
        kT = pers.tile([128, TB], F32)
        vTf = pers.tile([128, TB], F32)
        vT1 = pers.tile([64, TB], F32)
        Vsb = [[pers.tile([128, NKT * (HD + 1)], F32, name=f"Vsb{b}{h}")
                for h in range(2)] for b in range(B)]
        OT = [pers.tile([128, T], F32, name=f"OTb{b}") for b in range(B)]

        def qkv_chunk(xp, psqkv, tci):
            ts = slice(tci * QCH, (tci + 1) * QCH)
            xtiles = []
            for ct in range(NCT):
                xt = xp.tile([128, QCH], F32, tag="x", name=f"xt{tci}_{ct}")
                nc.sync.dma_start(
                    xt[:].bitcast(F32R),
                    xT[ct * 128:(ct + 1) * 128, ts].bitcast(F32R))
                xtiles.append(xt)
            for g in range(3):
                ps = psqkv.tile([128, QCH], F32, tag="ps", name=f"psq{tci}_{g}")
                for ct in range(NCT):
                    nc.tensor.matmul(
                        ps[:],
                        wsb[:, ct, g * 128:(g + 1) * 128].bitcast(F32R),
                        xtiles[ct][:].bitcast(F32R),
                        start=(ct == 0), stop=(ct == NCT - 1))
                if g == 0:
                    nc.scalar.activation(qT[:, ts].bitcast(F32R), ps[:],
                                         AF.Identity, bias=bqk_sb[:, 0:1])
                elif g == 1:
                    nc.scalar.activation(kT[:, ts].bitcast(F32R), ps[:],
                                         AF.Identity, bias=bqk_sb[:, 1:2])
                else:
                    nc.scalar.activation(vTf[:, ts], ps[:], AF.Copy)
                    nc.sync.dma_start(vT1[:, ts], vTf[64:128, ts])

        def rope_b(swpp, rtp, zt, b, nm):
            bs = slice(b * T, (b + 1) * T)
            swp = swpp.tile([128, T], F32, tag="swp", name=f"swp{nm}")
            for h in range(2):
                o = h * 64
                nc.sync.dma_start(swp[o:o + 32, :], zt[o + 32:o + 64, bs])
                nc.sync.dma_start(swp[o + 32:o + 64, :], zt[o:o + 32, bs])
            tmp = rtp.tile([128, T], F32, tag="rt", name=f"rt{nm}")
            nc.vector.tensor_mul(tmp[:], swp[:], sin_sb[:])
            nc.vector.tensor_mul(zt[:, bs].bitcast(F32R), zt[:, bs], cos_sb[:])
            nc.vector.tensor_add(zt[:, bs].bitcast(F32R), zt[:, bs], tmp[:])

        def vtrans_b(pstr, b):
            for h, vt in ((0, vTf), (1, vT1)):
                vov = Vsb[b][h][:].rearrange("p (t c) -> p t c", c=HD + 1)
                nc.vector.tensor_copy(vov[:, :, HD:HD + 1].bitcast(F32R),
                                      ones16[:])
                for tt in range(NKT):
                    pst = pstr.tile([128, 64], F32, tag="tr",
                                    name=f"pst{b}{h}{tt}")
                    nc.tensor.transpose(
                        pst[:],
                        vt[0:64, b * T + tt * 128: b * T + (tt + 1) * 128],
                        id64[:])
                    nc.vector.tensor_copy(
                        Vsb[b][h][:, tt * (HD + 1):tt * (HD + 1) + HD]
                        .bitcast(F32R),
                        pst[:])

        def attn_b(pools, b):
            pss, pso, ptp, mbp, smol, bcp = pools
            PIPE = 3
            for qc in range(NQC):
                nk = 4 * (qc + 1) if mode == "causal" else NKT
                qs = slice(b * T + qc * QCH, b * T + (qc + 1) * QCH)
                psO0 = pso.tile([65, QCH], F32, tag="o0", name=f"psO0_{b}{qc}")
                psO1 = pso.tile([65, QCH], F32, tag="o1", name=f"psO1_{b}{qc}")
                pts = {}

                def emit_pv(j, nk=nk, psO0=psO0, psO1=psO1, pts=pts):
                    st, sp = (j == 0), (j == nk - 1)
                    pt = pts.pop(j)
                    nc.tensor.matmul(
                        psO0[:],
                        Vsb[b][0][:, j * (HD + 1):(j + 1) * (HD + 1)]
                        .bitcast(F32R),
                        pt[:, 0:QCH].bitcast(F32R), start=st, stop=sp)
                    nc.tensor.matmul(
                        psO1[:],
                        Vsb[b][1][:, j * (HD + 1):(j + 1) * (HD + 1)]
                        .bitcast(F32R),
                        pt[:, QCH:2 * QCH].bitcast(F32R), start=st, stop=sp)

                for kt in range(nk):
                    ks = slice(b * T + kt * 128, b * T + (kt + 1) * 128)
                    psS = pss.tile([128, 2 * QCH], F32, tag="s",
                                   name=f"psS{b}{qc}{kt}")
                    nc.tensor.matmul(psS[:, 0:QCH],
                                     kT[0:64, ks].bitcast(F32R),
                                     qT[0:64, qs].bitcast(F32R),
                                     start=True, stop=True)
                    nc.tensor.matmul(psS[:, QCH:2 * QCH],
                                     kT[64:128, ks].bitcast(F32R),
                                     qT[64:128, qs].bitcast(F32R),
                                     start=True, stop=True)
                    pt = ptp.tile([128, 2 * QCH], F32, tag="pt",
                                  name=f"pt{b}{qc}{kt}")
                    nc.scalar.activation(pt[:].bitcast(F32R), psS[:], AF.Exp)
                    if mode == "causal" and kt >= 4 * qc:
                        base = qc * QCH - kt * 128
                        ptv = pt[:].rearrange("p (h q) -> p h q", q=QCH)
                        nc.gpsimd.affine_select(
                            out=ptv.bitcast(F32R), in_=ptv.bitcast(F32R),
                            compare_op=mybir.AluOpType.is_ge,
                            fill=0.0, base=base, channel_multiplier=-1,
                            pattern=[[0, 2], [1, QCH]])
                    elif mode == "bias":
                        mt = mbp.tile([128, QCH], F32, tag="mb",
                                      name=f"mt{b}{qc}{kt}")
                        nc.sync.dma_start(
                            mt[:], maskT[kt * 128:(kt + 1) * 128,
                                         qc * QCH:(qc + 1) * QCH])
                        nc.vector.tensor_mul(pt[:, 0:QCH].bitcast(F32R),
                                             pt[:, 0:QCH], mt[:])
                        nc.vector.tensor_mul(pt[:, QCH:2 * QCH].bitcast(F32R),
                                             pt[:, QCH:2 * QCH], mt[:])
                    pts[kt] = pt
                    if kt >= PIPE:
                        emit_pv(kt - PIPE)
                for j in range(max(0, nk - PIPE), nk):
                    emit_pv(j)

                # normalize + evict; sum(exp) in row 64 of psO*
                oqs = slice(qc * QCH, (qc + 1) * QCH)
                for h, psO in ((0, psO0), (1, psO1)):
                    nm = f"{b}{qc}{h}"
                    rw = smol.tile([65, QCH], F32, tag="rw", name=f"rw{nm}")
                    nc.scalar.activation(rw[64:65, :], psO[64:65, :], AF.Copy)
                    rz = smol.tile([1, QCH], F32, tag="rz", name=f"rz{nm}")
                    nc.sync.dma_start(rz[:], rw[64:65, :])
                    rr = smol.tile([1, QCH], F32, tag="rr", name=f"rr{nm}")
                    nc.vector.reciprocal_approx_fast(rr[:], rz[:])
                    bc = bcp.tile([128, QCH], F32, tag="bc", name=f"bc{nm}")
                    nc.gpsimd.partition_broadcast(bc[:], rr[:])
                    if h == 0:
                        nc.vector.tensor_mul(OT[b][0:64, oqs].bitcast(F32R),
                                             psO[0:64, :], bc[0:64, :])
                    else:
                        otmp = bcp.tile([64, QCH], F32, tag="otmp",
                                        name=f"otmp{nm}")
                        nc.vector.tensor_mul(otmp[:], psO[0:64, :],
                                             bc[0:64, :])
                        nc.sync.dma_start(OT[b][64:128, oqs].bitcast(F32R),
                                          otmp[:].bitcast(F32R))

        def proj_b(psy, ybp, b):
            for tt in range(NKT):
                for ncol in range(2):
                    nm = f"{b}{tt}{ncol}"
                    ps = psy.tile([128, QCH], F32, tag="y", name=f"psy{nm}")
                    nc.tensor.matmul(
                        ps[:],
                        OT[b][:, tt * 128:(tt + 1) * 128].bitcast(F32R),
                        wo_sb[:, ncol * QCH:(ncol + 1) * QCH].bitcast(F32R),
                        start=True, stop=True)
                    yb = ybp.tile([128, QCH], F32, tag="yb", name=f"yb{nm}")
                    nc.any.tensor_copy(yb[:], ps[:])
                    nc.sync.dma_start(
                        y[b * T + tt * 128: b * T + (tt + 1) * 128,
                          ncol * QCH:(ncol + 1) * QCH], yb[:])

        # ---- phases, interleaved per batch ----
        with tc.tile_pool(name="xp", bufs=16) as xp, \
             tc.tile_pool(name="psqkv", bufs=3, space="PSUM") as psqkv, \
             tc.tile_pool(name="pstr", bufs=2, space="PSUM") as pstr, \
             tc.tile_pool(name="swp", bufs=2) as swpp, \
             tc.tile_pool(name="rtmp", bufs=2) as rtp:
            for tci in range(NTC // 2):
                qkv_chunk(xp, psqkv, tci)
            rope_b(swpp, rtp, qT, 0, "q0")
            rope_b(swpp, rtp, kT, 0, "k0")
            vtrans_b(pstr, 0)
            for tci in range(NTC // 2, NTC):
                qkv_chunk(xp, psqkv, tci)
            rope_b(swpp, rtp, qT, 1, "q1")
            rope_b(swpp, rtp, kT, 1, "k1")
            vtrans_b(pstr, 1)

        for b in range(B):
            with tc.tile_pool(name="pss", bufs=2, space="PSUM") as pss, \
                 tc.tile_pool(name="pso", bufs=2, space="PSUM") as pso, \
                 tc.tile_pool(name="ptp", bufs=5) as ptp, \
                 tc.tile_pool(name="mbp", bufs=4) as mbp, \
                 tc.tile_pool(name="smol", bufs=4) as smol, \
                 tc.tile_pool(name="bcp", bufs=4) as bcp:
                attn_b((pss, pso, ptp, mbp, smol, bcp), b)
            with tc.tile_pool(name="psy", bufs=4, space="PSUM") as psy, \
                 tc.tile_pool(name="ybp", bufs=4) as ybp:
                proj_b(psy, ybp, b)


def _build_program(mode):
    if mode in _PROG_CACHE:
        return _PROG_CACHE[mode]
    nc = bacc.Bacc("TRN2", target_bir_lowering=False, debug=False,
                   num_devices=NCORES)
    dram = {
        "xT": nc.dram_tensor("xT", [C, TB], F32, kind="ExternalInput").ap(),
        "wT": nc.dram_tensor("wT", [C, 384], F32, kind="ExternalInput").ap(),
        "bqk": nc.dram_tensor("bqk", [128, 2], F32, kind="ExternalInput").ap(),
        "cosT": nc.dram_tensor("cosT", [128, T], F32, kind="ExternalInput").ap(),
        "sinS": nc.dram_tensor("sinS", [128, T], F32, kind="ExternalInput").ap(),
        "woT": nc.dram_tensor("woT", [128, C], F32, kind="ExternalInput").ap(),
        "y": nc.dram_tensor("y", [TB, C], F32, kind="ExternalOutput").ap(),
    }
    if mode == "bias":
        dram["maskT"] = nc.dram_tensor("maskT", [T, T], F32,
                                       kind="ExternalInput").ap()
    with tile.TileContext(nc) as tc:
        _emit(tc, mode, dram)
    nc.compile()
    _PROG_CACHE[mode] = (nc, dram)
    return nc, dram


def _rope_tables():
    inv_freq = 1.0 / (10000.0 ** (np.arange(0, HD, 2, dtype=np.float64) / HD))
    freqs = np.arange(T, dtype=np.float64)[:, None] * inv_freq[None, :]
    cos = np.concatenate([np.cos(freqs), np.cos(freqs)], axis=-1)  # [T, 64]
    sin = np.concatenate([np.sin(freqs), np.sin(freqs)], axis=-1)
    cE = cos[:, 0::2].T  # [32, T] rows i -> dim 2i
    cO = cos[:, 1::2].T
    sE = sin[:, 0::2].T
    sO = sin[:, 1::2].T
    cosT = np.concatenate([cE, cO, cE, cO], axis=0).astype(np.float32)
    sinS = np.concatenate([-sE, sO, -sE, sO], axis=0).astype(np.float32)
    return np.ascontiguousarray(cosT), np.ascontiguousarray(sinS)


def kernel(x, mask, Wqkv, bqkv, Wo, bo):
    x = np.asarray(x, dtype=np.float32)
    mask = np.asarray(mask)
    Wqkv = np.asarray(Wqkv, dtype=np.float32)
    bqkv = np.asarray(bqkv, dtype=np.float32)
    Wo = np.asarray(Wo, dtype=np.float32)
    bo = np.asarray(bo, dtype=np.float32)

    mb = mask.reshape(T, T)
    if np.array_equal(mb != 0, np.tril(np.ones((T, T), dtype=bool))):
        mode = "causal"
    elif np.all(mb != 0):
        mode = "dense"
    else:
        mode = "bias"

    nc, dram = _build_program(mode)

    xTn = np.ascontiguousarray(x.reshape(TB, C).T)
    cosT, sinS = _rope_tables()
    scale = 1.0 / np.sqrt(np.float32(HD))

    evens = np.arange(0, HD, 2)
    odds = evens + 1

    in_maps = []
    for c in range(NCORES):
        h0, h1 = 2 * c, 2 * c + 1
        qrows = np.concatenate([h0 * HD + evens, h0 * HD + odds,
                                h1 * HD + evens, h1 * HD + odds])
        krows = C + qrows
        vrows = np.concatenate([2 * C + h0 * HD + np.arange(HD),
                                2 * C + h1 * HD + np.arange(HD)])
        wq = Wqkv[qrows, :] * scale
        wk = Wqkv[krows, :]
        wv = Wqkv[vrows, :]
        wT = np.ascontiguousarray(np.concatenate([wq, wk, wv], axis=0).T)
        bqk = np.stack([bqkv[qrows] * scale, bqkv[krows]], axis=1)
        woT = np.ascontiguousarray(Wo[:, 128 * c:128 * (c + 1)].T)
        im = {
            "xT": xTn, "wT": wT,
            "bqk": np.ascontiguousarray(bqk, dtype=np.float32),
            "cosT": cosT, "sinS": sinS, "woT": woT,
        }
        if mode == "bias":
            im["maskT"] = np.ascontiguousarray(
                (mb != 0).astype(np.float32).T)
        in_maps.append(im)

    res = run_bass_kernel_spmd(nc, in_maps, core_ids=list(range(NCORES)))
    y = np.zeros((TB, C), dtype=np.float32)
    for c in range(NCORES):
        y += res.results[c]["y"]
    bv = bqkv[2 * C:3 * C]
    y += (bo + bv @ Wo.T)[None, :]
    return y.reshape(B, T, C)
